# revision 18
# baseline (speedup 1.0000x reference)
import numpy as np
import ml_dtypes

bf16 = ml_dtypes.bfloat16

B, NX, NT, W, M = 4, 512, 128, 64, 16
N = NX
EPS = 1e-5
TWO_PI = float(2.0 * np.pi)
INV_2PI = float(1.0 / (2.0 * np.pi))
MAGIC = float(1.5 * 2 ** 23)
N_CORES = 8
WWARM = 24
T1 = (NT + WWARM) // 2  # 76; chain A: t 0..T1-1, chain B: warm T1-WWARM..T1-1, real T1..NT-1
NSEQ = 256              # sequences per core in launch 2


# ----------------------------------------------------------------------------
# toolchain workaround: this walrus build rejects instructions carrying more
# sync waits than the ISA struct holds (the Tile tail-drain gets one wait per
# engine/DMA queue). Hoist excess waits into single-wait NoOps just before.
# ----------------------------------------------------------------------------
def _legalize_multiwaits(nc):
    import concourse.mybir as mybir
    for f in nc.m.functions:
        for blk in f.blocks:
            insts = blk.instructions
            out = []
            changed = False
            for inst in insts:
                si = inst.sync_info
                limit = 2 if isinstance(inst, mybir.InstEventSemaphore) else 1
                if si is not None and len(si.on_wait) > limit:
                    waits = list(si.on_wait)
                    head, tail = waits[:-limit], waits[-limit:]
                    for k, w in enumerate(head):
                        out.append(mybir.InstNoOp(
                            name=f"{inst.name}-wsplit{k}", ins=[], outs=[],
                            engine=inst.engine,
                            sync_info=mybir.SyncInfo(on_wait=[w], on_update=[])))
                    inst.sync_info = mybir.SyncInfo(
                        on_wait=tail, on_update=list(si.on_update))
                    changed = True
                out.append(inst)
            if changed:
                blk.instructions = out


# ----------------------------------------------------------------------------
# host parameter precompute (pure numpy, cheap)
# ----------------------------------------------------------------------------
def _host_params(grid, pole_re, pole_im, res_re, res_im, conv_w, conv_b):
    g64 = grid.astype(np.float64)
    dt = g64[1] - g64[0]
    f = np.fft.fftfreq(N, dt)
    lam = 1j * 2 * np.pi * f
    w_vec = np.zeros(N, complex)
    w_vec[1:] = 1.0 / lam[1:]
    Kc = np.fft.ifft(w_vec)
    K_re = Kc.real
    altn = np.where(np.arange(N) % 2 == 0, 1.0, -1.0)
    kappa = float((Kc.imag * altn).mean())
    idx = (np.arange(N)[None, :] - np.arange(N)[:, None]) % N
    C = K_re[idx]                              # [n', n]
    s_re = N * K_re[(-np.arange(N)) % N]
    Ssum = float(s_re.sum())

    p = (pole_re + 1j * pole_im).astype(np.complex128)
    r = (res_re + 1j * res_im).astype(np.complex128)
    R1 = r.sum(-1)

    Eall = np.zeros((128, 256))
    for k in range(3):
        pk = p ** k / (1.0 if k < 2 else 2.0) if k else np.ones_like(p)
        Ak, Bk = pk.real, pk.imag
        Mre = np.einsum('jcm,com->jo', r.real, Ak) - np.einsum('jcm,com->jo', r.imag, Bk)
        Mim = -np.einsum('jcm,com->jo', r.imag, Ak) - np.einsum('jcm,com->jo', r.real, Bk)
        Eall[0:64, 64 * k:64 * k + 64] = -Mre / N
        Eall[64:128, 64 * k:64 * k + 64] = -kappa * Mim
    Eall[64:128, 192:256] = -kappa * R1.imag

    G4 = np.stack([np.ones(N), g64, g64 ** 2, altn]).astype(bf16)   # [4, N]

    blkdiag = lambda A: np.block([[A, np.zeros_like(A)], [np.zeros_like(A), A]])
    r1blk = blkdiag(R1.real).astype(bf16)                 # [(t,i), (t,o)]
    convblk = blkdiag(conv_w.T / TWO_PI).astype(bf16)     # lhsT: [i, o] blocks
    sa4 = np.zeros((N, 4), np.float32)
    sa4[:, 0] = s_re
    sa4[:, 1] = altn
    cb2pi = np.tile(conv_b.astype(np.float64) / TWO_PI, 2).reshape(128, 1)

    return dict(
        cmat=C.astype(bf16), r1blk=r1blk, convblk=convblk,
        sa4=sa4.astype(bf16), eall=Eall.astype(np.float32),
        g4=np.ascontiguousarray(G4), Ssum=Ssum,
        cb2pi=cb2pi.astype(np.float32),
        identb=np.eye(128, dtype=bf16), identf=np.eye(128, dtype=np.float32),
        epsv=np.full((128, 1), EPS, np.float32),
    )


# ----------------------------------------------------------------------------
# launch 1: spectral + conv + sin   (per core: one (b, t-half) slab)
# ----------------------------------------------------------------------------
def _build_l1(Ssum):
    import concourse.bass as bass
    import concourse.mybir as mybir
    from concourse.tile import TileContext

    fp32 = mybir.dt.float32
    bft = mybir.dt.bfloat16
    AO = mybir.AluOpType
    AF = mybir.ActivationFunctionType

    nc = bass.Bass()
    xin_d = nc.declare_dram_parameter("xin", [512, 4096], bft, isOutput=False)
    cmat_d = nc.declare_dram_parameter("cmat", [512, 512], bft, isOutput=False)
    r1blk_d = nc.declare_dram_parameter("r1blk", [128, 128], bft, isOutput=False)
    convblk_d = nc.declare_dram_parameter("convblk", [128, 128], bft, isOutput=False)
    sa4_d = nc.declare_dram_parameter("sa4", [512, 4], bft, isOutput=False)
    eall_d = nc.declare_dram_parameter("eall", [128, 256], fp32, isOutput=False)
    g4_d = nc.declare_dram_parameter("g4", [4, 512], bft, isOutput=False)
    identb_d = nc.declare_dram_parameter("identb", [128, 128], bft, isOutput=False)
    identf_d = nc.declare_dram_parameter("identf", [128, 128], fp32, isOutput=False)
    cb2pi_d = nc.declare_dram_parameter("cb2pi", [128, 1], fp32, isOutput=False)
    epsv_d = nc.declare_dram_parameter("epsv", [128, 1], fp32, isOutput=False)
    hout_d = nc.declare_dram_parameter("hout", [64, 64, 512], bft, isOutput=True)

    with TileContext(nc) as tc:
        with tc.tile_pool(name="const", bufs=1) as cp, \
             tc.tile_pool(name="xa", bufs=1) as xap, \
             tc.tile_pool(name="big", bufs=1) as bigp, \
             tc.tile_pool(name="st", bufs=1) as stp, \
             tc.tile_pool(name="wk", bufs=3) as wkp, \
             tc.tile_pool(name="scr", bufs=2) as scrp, \
             tc.tile_pool(name="ps", bufs=8, space="PSUM") as ps:

            # ---- constants in ----
            cm = [cp.tile([128, 512], bft, tag=f"cm{c}", name=f"cm{c}") for c in range(4)]
            for c in range(4):
                nc.sync.dma_start(out=cm[c][:], in_=cmat_d[128 * c:128 * (c + 1), :])
            r1blk = cp.tile([128, 128], bft)
            nc.sync.dma_start(out=r1blk[:], in_=r1blk_d[:])
            convblk = cp.tile([128, 128], bft)
            nc.sync.dma_start(out=convblk[:], in_=convblk_d[:])
            sa4 = [cp.tile([128, 4], bft, tag=f"sa{c}", name=f"sa{c}") for c in range(4)]
            for c in range(4):
                nc.sync.dma_start(out=sa4[c][:], in_=sa4_d[128 * c:128 * (c + 1), :])
            eall = cp.tile([128, 256], fp32)
            nc.sync.dma_start(out=eall[:], in_=eall_d[:])
            g4 = cp.tile([4, 512], bft)
            nc.sync.dma_start(out=g4[:], in_=g4_d[:])
            identb = cp.tile([128, 128], bft)
            nc.sync.dma_start(out=identb[:], in_=identb_d[:])
            identf = cp.tile([128, 128], fp32)
            nc.sync.dma_start(out=identf[:], in_=identf_d[:])
            cb2pi = cp.tile([128, 1], fp32)
            nc.sync.dma_start(out=cb2pi[:], in_=cb2pi_d[:])
            epsv = cp.tile([128, 1], fp32)
            nc.sync.dma_start(out=epsv[:], in_=epsv_d[:])

            # ---- x in (bf16, layout A: [nx, (t,w)]) ----
            xa = [xap.tile([128, 4096], bft, tag=f"xa{c}", name=f"xa{c}") for c in range(4)]
            for c in range(4):
                nc.sync.dma_start(out=xa[c][:], in_=xin_d[128 * c:128 * (c + 1), :])

            # ---- S/alt matvec: out rows [Sx, Xalt, -, -] ----
            sxrow = stp.tile([4, 4096], fp32)
            for ch in range(8):
                pssx = ps.tile([4, 512], fp32, tag="ps", name="pssx")
                for c in range(4):
                    nc.tensor.matmul(pssx[:], sa4[c][:],
                                     xa[c][:, 512 * ch:512 * (ch + 1)],
                                     start=(c == 0), stop=(c == 3))
                nc.vector.tensor_copy(sxrow[:, 512 * ch:512 * (ch + 1)], pssx[:])

            # ---- transpose to layout B + norm1 stats + drains (groups of 8) ----
            ub = [bigp.tile([128, 512], bft, tag=f"ub{j}", name=f"ub{j}") for j in range(32)]
            xr = [bigp.tile([128, 512], bft, tag=f"xr{j}", name=f"xr{j}") for j in range(32)]
            sum1 = stp.tile([128, 32], fp32)
            ssq1 = stp.tile([128, 32], fp32)
            statpack = stp.tile([128, 64], fp32)   # cols 0:32 rs1, 32:64 negms1
            rs1 = statpack[:, 0:32]
            negms1 = statpack[:, 32:64]
            mean1 = stp.tile([128, 32], fp32)
            tmpa = stp.tile([128, 32], fp32)

            for g in range(4):
                js = range(8 * g, 8 * g + 8)
                ptw = {}
                for j in js:
                    ptw[j] = ps.tile([128, 512], bft, tag="ps", name=f"ptw{j}")
                    for c in range(4):
                        nc.tensor.transpose(ptw[j][:, 128 * c:128 * (c + 1)],
                                            xa[c][:, 128 * j:128 * (j + 1)],
                                            identb[:])
                for j in js:
                    nc.vector.tensor_scalar(
                        xr[j][:], ptw[j][:], 1.0, 0.0, AO.mult, AO.add,
                        accum_out=sum1[:, j:j + 1])
                    sq = scrp.tile([128, 512], bft, tag="sq", name="sq")
                    nc.scalar.activation(sq[:], ptw[j][:], AF.Square,
                                         accum_out=ssq1[:, j:j + 1])
                gs = slice(8 * g, 8 * g + 8)
                nc.vector.tensor_scalar(mean1[:, gs], sum1[:, gs],
                                        1.0 / 512.0, None, AO.mult)
                nc.vector.scalar_tensor_tensor(tmpa[:, gs], mean1[:, gs], 1.0,
                                               mean1[:, gs], AO.mult, AO.mult)
                nc.vector.scalar_tensor_tensor(tmpa[:, gs], ssq1[:, gs],
                                               1.0 / 512.0, tmpa[:, gs],
                                               AO.mult, AO.subtract)
                nc.scalar.activation(tmpa[:, gs], tmpa[:, gs], AF.Sqrt,
                                     bias=epsv[:, 0:1])
                nc.vector.reciprocal(rs1[:, gs], tmpa[:, gs])
                nc.vector.scalar_tensor_tensor(negms1[:, gs], mean1[:, gs], -1.0,
                                               rs1[:, gs], AO.mult, AO.mult)
                for j in js:
                    nc.scalar.activation(ub[j][:], ptw[j][:], AF.Identity,
                                         bias=negms1[:, j:j + 1],
                                         scale=rs1[:, j:j + 1])

            # ---- S-fix and D coefficients ----
            pst = ps.tile([64, 128], fp32, tag="ps", name="pst")
            nc.tensor.transpose(pst[:], statpack[:], identf[:])
            statT = stp.tile([64, 128], fp32)
            nc.vector.tensor_copy(statT[:], pst[:])

            rs1m2 = stp.tile([64, 128], fp32)
            negms2m = stp.tile([64, 128], fp32)
            # statT[q, p] = statpack[p, q]: q<32 -> rs1 tile q; q>=32 -> negms tile q-32
            # rs1m[t=2q+tt, w] = statT[q, 64*tt + w]
            for half in range(2):
                nc.sync.dma_start(
                    out=rs1m2[:, 64 * half:64 * half + 64],
                    in_=statT[0:32, :].rearrange("q (tt w) -> q tt w", tt=2))
            nc.sync.dma_start(
                out=negms2m[:, 0:64],
                in_=statT[32:64, :].rearrange("q (tt w) -> q tt w", tt=2))
            nc.vector.memset(negms2m[:, 64:128], 0.0)

            Sm = stp.tile([64, 128], fp32)
            nc.sync.dma_start(out=Sm[:, 0:64],
                              in_=sxrow[0:1, :].rearrange("o (t j) -> o t j", t=64))
            nc.sync.dma_start(out=Sm[:, 64:128],
                              in_=sxrow[1:2, :].rearrange("o (t j) -> o t j", t=64))
            nc.vector.tensor_tensor(out=Sm[:], in0=Sm[:], in1=rs1m2[:], op=AO.mult)
            nc.vector.scalar_tensor_tensor(Sm[:], negms2m[:], Ssum, Sm[:],
                                           AO.mult, AO.add)
            psu = ps.tile([128, 64], fp32, tag="ps", name="psu")
            nc.tensor.transpose(psu[:], Sm[:], identf[0:64, 0:64])
            suT = stp.tile([128, 64], fp32)
            nc.vector.tensor_copy(suT[:], psu[:])
            psd = ps.tile([64, 256], fp32, tag="ps", name="psd")
            nc.tensor.matmul(psd[:], suT[:], eall[:], start=True, stop=True)
            dallf = stp.tile([64, 256], fp32)
            nc.vector.tensor_copy(dallf[:], psd[:])
            dallb = stp.tile([64, 256], bft)
            nc.vector.tensor_copy(dallb[:], dallf[:])
            dco = stp.tile([4, 4096], bft)
            for k in range(4):
                nc.sync.dma_start(
                    out=dco[k:k + 1, :],
                    in_=dallb[:, 64 * k:64 * k + 64])

            # ---- channel mix (P^T direct) ----
            pt = [bigp.tile([128, 4096], bft, tag=f"pt{c}", name=f"pt{c}") for c in range(4)]
            for c in range(4):
                for jq in range(8):
                    psc = ps.tile([128, 512], fp32, tag="ps", name="psc")
                    for jj in range(4):
                        j = 4 * jq + jj
                        nc.tensor.matmul(psc[:, 128 * jj:128 * (jj + 1)],
                                         ub[j][:, 128 * c:128 * (c + 1)],
                                         r1blk[:], start=True, stop=True)
                    nc.vector.tensor_copy(pt[c][:, 512 * jq:512 * (jq + 1)], psc[:])

            # ---- time mix + rank + conv + norm2 + sin (groups of 6) ----
            mv2 = stp.tile([128, 64], fp32)      # (mean, var) pairs per tile
            rs2p = stp.tile([128, 32], fp32)
            cc2 = stp.tile([128, 32], fp32)
            tmpb = stp.tile([128, 32], fp32)
            groups = [list(range(6 * g, min(6 * g + 6, 32))) for g in range(6)]
            for js in groups:
                px1 = {}
                for j in js:
                    px1[j] = ps.tile([128, 512], fp32, tag="ps", name=f"px1_{j}")
                    for c in range(4):
                        nc.tensor.matmul(px1[j][:], pt[c][:, 128 * j:128 * (j + 1)],
                                         cm[c][:], start=(c == 0), stop=False)
                    nc.tensor.matmul(px1[j][:], dco[:, 128 * j:128 * (j + 1)],
                                     g4[:], start=False, stop=True)
                    bns = scrp.tile([128, 6], fp32, tag="bns", name="bns")
                    nc.vector.bn_stats(bns[:], px1[j][:])
                    nc.vector.bn_aggr(mv2[:, 2 * j:2 * j + 2], bns[:])
                g0, g1 = js[0], js[-1] + 1
                gs = slice(g0, g1)
                nc.scalar.activation(tmpb[:, gs], mv2[:, 2 * g0 + 1:2 * g1:2],
                                     AF.Sqrt, bias=epsv[:, 0:1])
                nc.vector.reciprocal(tmpb[:, gs], tmpb[:, gs])
                nc.vector.tensor_scalar(rs2p[:, gs], tmpb[:, gs], INV_2PI, None,
                                        AO.mult)
                nc.vector.scalar_tensor_tensor(tmpb[:, gs], mv2[:, 2 * g0:2 * g1:2],
                                               -1.0, rs2p[:, gs], AO.mult, AO.mult)
                nc.vector.tensor_scalar(cc2[:, gs], tmpb[:, gs], cb2pi[:, 0:1],
                                        None, AO.add)
                for j in js:
                    pcv = ps.tile([128, 512], fp32, tag="ps", name="pcv")
                    nc.tensor.matmul(pcv[:], convblk[:], xr[j][:],
                                     start=True, stop=True)
                    qa = wkp.tile([128, 512], fp32, tag="qa", name="qa")
                    nc.scalar.activation(qa[:], px1[j][:], AF.Identity,
                                         bias=cc2[:, j:j + 1],
                                         scale=rs2p[:, j:j + 1])
                    qq = wkp.tile([128, 512], fp32, tag="qq", name="qq")
                    nc.vector.scalar_tensor_tensor(qq[:], pcv[:], 1.0, qa[:],
                                                   AO.mult, AO.add)
                    kk = wkp.tile([128, 512], fp32, tag="kk", name="kk")
                    nc.gpsimd.tensor_scalar(kk[:], qq[:], MAGIC, MAGIC,
                                            AO.add, AO.subtract)
                    rr = wkp.tile([128, 512], fp32, tag="rr", name="rr")
                    nc.gpsimd.tensor_tensor(out=rr[:], in0=qq[:], in1=kk[:],
                                            op=AO.subtract)
                    ht = wkp.tile([128, 512], bft, tag="ht", name="ht")
                    nc.scalar.activation(ht[:], rr[:], AF.Sin, scale=TWO_PI)
                    nc.sync.dma_start(out=hout_d[2 * j:2 * j + 2, :, :], in_=ht[:])

    _legalize_multiwaits(nc)
    return nc


_L1_CACHE = {}


def _get_l1(Ssum):
    key = round(Ssum, 9)
    if key not in _L1_CACHE:
        _L1_CACHE[key] = _build_l1(Ssum)
    return _L1_CACHE[key]


_LAST_PERF = {}


def _run_l1(x, prm):
    import os
    from concourse.bass_utils import run_bass_kernel_spmd
    nc = _get_l1(prm['Ssum'])
    in_maps = []
    xb = x.astype(bf16)
    for c in range(N_CORES):
        b, th = c // 2, c % 2
        slab = np.ascontiguousarray(xb[b, :, 64 * th:64 * (th + 1), :]).reshape(512, 4096)
        m = {"xin": slab}
        for k in ("cmat", "r1blk", "convblk", "sa4", "eall", "g4", "identb",
                  "identf", "cb2pi", "epsv"):
            m[k] = prm[k]
        in_maps.append(m)
    trace = bool(os.environ.get("TRNK_TRACE"))
    res = run_bass_kernel_spmd(nc, in_maps, list(range(N_CORES)), trace=trace)
    _LAST_PERF["l1_ns"] = res.exec_time_ns
    _LAST_PERF["l1_trace"] = res.instructions_and_trace[1] if res.instructions_and_trace else None
    return [r["hout"] for r in res.results]


# ----------------------------------------------------------------------------
# reference-path helpers for the final host fallback (and debugging)
# ----------------------------------------------------------------------------
def _instance_norm(v):
    mu = v.mean(-1, keepdims=True)
    var = ((v - mu) ** 2).mean(-1, keepdims=True)
    return (v - mu) / np.sqrt(var + EPS)


def _host_launch1(x, prm, conv_w, conv_b):
    xs = np.ascontiguousarray(x.transpose(0, 2, 3, 1)).reshape(B * NT, W, NX)
    xb = xs.astype(bf16).astype(np.float32)
    mu = xb.mean(-1, keepdims=True)
    var = (xb ** 2).mean(-1, keepdims=True) - mu ** 2
    rs = 1.0 / np.sqrt(var + EPS)
    u = ((xb - mu) * rs).astype(bf16).astype(np.float32)
    R1b = prm['r1blk'][:64, :64].astype(np.float32)
    PT = np.einsum('bin,io->bon', u, R1b).astype(bf16).astype(np.float32)
    x1a = np.einsum('bon,nz->boz', PT, prm['cmat'].astype(np.float32))
    sre = prm['sa4'][:, 0].astype(np.float32)
    alt = prm['sa4'][:, 1].astype(np.float32)
    Sx = np.einsum('bjn,n->bj', xb, sre)
    Xa = np.einsum('bjn,n->bj', xb, alt)
    S = rs[:, :, 0] * Sx + prm['Ssum'] * (-mu[:, :, 0] * rs[:, :, 0])
    ua = rs[:, :, 0] * Xa
    SU = np.concatenate([S, ua], 1)
    Dall = (SU @ prm['eall']).astype(bf16).astype(np.float32)
    G4 = prm['g4'].astype(np.float32)
    x2 = np.einsum('btk,kn->btn',
                   Dall.reshape(-1, 4, 64).transpose(0, 2, 1), G4)
    x1f = x1a + x2
    mu2 = x1f.mean(-1, keepdims=True)
    var2 = (x1f ** 2).mean(-1, keepdims=True) - mu2 ** 2
    rs2 = 1.0 / np.sqrt(var2 + EPS)
    convb = np.einsum('oi,bin->bon',
                      prm['convblk'][:64, :64].T.astype(np.float32), xb)
    qa = x1f * (rs2 * INV_2PI) + (-mu2 * rs2 + conv_b[None, :, None]) * INV_2PI
    q = (qa + convb).astype(np.float32)
    kq = ((q + np.float32(MAGIC)) - np.float32(MAGIC)).astype(np.float32)
    h = np.sin(TWO_PI * (q - kq)).astype(bf16).astype(np.float32)
    return h  # [BT, W, NX]


def kernel(x, grid, pole_re, pole_im, res_re, res_im, conv_w, conv_b,
           gru_Wx, gru_Wh, gru_bx, gru_bh, out_W, out_b):
    x = np.asarray(x, np.float32)
    grid = np.asarray(grid, np.float32)
    prm = _host_params(grid, np.asarray(pole_re), np.asarray(pole_im),
                       np.asarray(res_re), np.asarray(res_im),
                       np.asarray(conv_w, np.float32), np.asarray(conv_b, np.float32))
    houts = _run_l1(x, prm)   # 8 x [64, 64, 512] bf16
    y = _run_l2(houts, np.asarray(gru_Wx, np.float32), np.asarray(gru_Wh, np.float32),
                np.asarray(gru_bx, np.float32), np.asarray(gru_bh, np.float32),
                np.asarray(out_W, np.float32), np.asarray(out_b, np.float32))
    return y


# ----------------------------------------------------------------------------
# launch 2: GRU (2 time-chunks, alternating) + output projection
# ----------------------------------------------------------------------------
def _build_l2():
    import concourse.bass as bass
    import concourse.mybir as mybir
    from concourse.tile import TileContext

    fp32 = mybir.dt.float32
    bft = mybir.dt.bfloat16
    AO = mybir.AluOpType
    AF = mybir.ActivationFunctionType
    T = NT
    TOFF = T - T1  # 52: chain-B time offset

    nc = bass.Bass()
    hin_d = nc.declare_dram_parameter("hin", [128, 64, 256], bft, isOutput=False)
    wxrz_d = nc.declare_dram_parameter("wxrz", [64, 128], bft, isOutput=False)
    wxn_d = nc.declare_dram_parameter("wxn", [64, 64], bft, isOutput=False)
    whrz_d = nc.declare_dram_parameter("whrz", [64, 128], bft, isOutput=False)
    whn_d = nc.declare_dram_parameter("whn", [64, 64], bft, isOutput=False)
    bhn_d = nc.declare_dram_parameter("bhn", [1, 64], bft, isOutput=False)
    ones_d = nc.declare_dram_parameter("ones512", [1, 512], bft, isOutput=False)
    brz_d = nc.declare_dram_parameter("brz", [128, 1], fp32, isOutput=False)
    bxn_d = nc.declare_dram_parameter("bxncol", [64, 1], fp32, isOutput=False)
    outwt_d = nc.declare_dram_parameter("outwt", [64, 64], bft, isOutput=False)
    outbrep_d = nc.declare_dram_parameter("outbrep", [128, 512], fp32, isOutput=False)
    yout_d = nc.declare_dram_parameter("yout", [256, 128, 64], fp32, isOutput=True)

    with TileContext(nc) as tc:
        with tc.tile_pool(name="const", bufs=1) as cp, \
             tc.tile_pool(name="hs", bufs=1) as hsp, \
             tc.tile_pool(name="gw", bufs=2) as gwp, \
             tc.tile_pool(name="ps", bufs=8, space="PSUM") as ps:

            wxrz = cp.tile([64, 128], bft)
            nc.sync.dma_start(out=wxrz[:], in_=wxrz_d[:])
            wxn = cp.tile([64, 64], bft)
            nc.sync.dma_start(out=wxn[:], in_=wxn_d[:])
            whrz = cp.tile([64, 128], bft)
            nc.sync.dma_start(out=whrz[:], in_=whrz_d[:])
            whn = cp.tile([64, 64], bft)
            nc.sync.dma_start(out=whn[:], in_=whn_d[:])
            bhn = cp.tile([1, 64], bft)
            nc.sync.dma_start(out=bhn[:], in_=bhn_d[:])
            ones512 = cp.tile([1, 512], bft)
            nc.sync.dma_start(out=ones512[:], in_=ones_d[:])
            brz = cp.tile([128, 1], fp32)
            nc.sync.dma_start(out=brz[:], in_=brz_d[:])
            bxnc = cp.tile([64, 1], fp32)
            nc.sync.dma_start(out=bxnc[:], in_=bxn_d[:])
            outwt = cp.tile([64, 64], bft)
            nc.sync.dma_start(out=outwt[:], in_=outwt_d[:])
            outbrep = cp.tile([128, 512], fp32)
            nc.sync.dma_start(out=outbrep[:], in_=outbrep_d[:])

            NS = 56  # steps per pair
            hsA = hsp.tile([64, 512 * NS], bft)   # pair A = (c1: t=8+i, c3: t=72+i)
            hsB = hsp.tile([64, 512 * NS], bft)   # pair B = (c0: t=i-24, c2: t=40+i)
            hz = cp.tile([64, 512], bft)
            nc.vector.memset(hz[:], 0.0)

            with tc.tile_pool(name="hin", bufs=1) as hinp:
                hin_all = hinp.tile([64, 32768], bft)
                nc.sync.dma_start(
                    out=hin_all[:].rearrange("w (t n) -> w t n", t=128),
                    in_=hin_d[:].rearrange("t w n -> w t n"))

                def emit(pid, i, hsX, tA, tB):
                    if i == 0:
                        hpA, hpB = hz[:, 0:256], hz[:, 256:512]
                        hp_full = hz[:]
                    else:
                        base = (i - 1) * 512
                        hpA = hsX[:, base:base + 256]
                        hpB = hsX[:, base + 256:base + 512]
                        hp_full = hsX[:, base:base + 512]
                    hA = hin_all[:, tA * 256:(tA + 1) * 256]
                    hB = hin_all[:, tB * 256:(tB + 1) * 256]
                    psA = ps.tile([128, 512], fp32, tag="ps", name=f"psA{pid}_{i}")
                    # group-by-group: a start=True clears the whole bank's
                    # has_written bits, so finish half A before starting half B
                    nc.tensor.matmul(psA[:, 0:256], whrz[:], hpA, start=True, stop=False)
                    nc.tensor.matmul(psA[:, 0:256], wxrz[:], hA, start=False, stop=True)
                    nc.tensor.matmul(psA[:, 256:512], whrz[:], hpB, start=True, stop=False)
                    nc.tensor.matmul(psA[:, 256:512], wxrz[:], hB, start=False, stop=True)
                    rzt = gwp.tile([128, 512], bft, tag=f"rzt{pid}", name=f"rzt{pid}_{i}")
                    nc.scalar.activation(rzt[:], psA[:], AF.Sigmoid, bias=brz[:, 0:1])
                    psB = ps.tile([64, 512], fp32, tag="ps", name=f"psB{pid}_{i}")
                    nc.tensor.matmul(psB[:], bhn[:], ones512[:], start=True, stop=False)
                    nc.tensor.matmul(psB[:, 0:256], whn[:], hpA, start=False, stop=True)
                    nc.tensor.matmul(psB[:, 256:512], whn[:], hpB, start=False, stop=True)
                    psC = ps.tile([64, 512], fp32, tag="ps", name=f"psC{pid}_{i}")
                    nc.tensor.matmul(psC[:, 0:256], wxn[:], hA, start=True, stop=True)
                    nc.tensor.matmul(psC[:, 256:512], wxn[:], hB, start=True, stop=True)
                    t1 = gwp.tile([64, 512], bft, tag=f"t1{pid}", name=f"t1_{pid}_{i}")
                    nc.vector.tensor_tensor(out=t1[:], in0=rzt[64:128, :], in1=psB[:],
                                            op=AO.mult)
                    npre = gwp.tile([64, 512], bft, tag=f"np{pid}", name=f"np{pid}_{i}")
                    nc.vector.tensor_tensor(out=npre[:], in0=t1[:], in1=psC[:],
                                            op=AO.add)
                    nt_ = gwp.tile([64, 512], bft, tag=f"nt{pid}", name=f"nt{pid}_{i}")
                    nc.scalar.activation(nt_[:], npre[:], AF.Tanh, bias=bxnc[:, 0:1])
                    d_ = gwp.tile([64, 512], bft, tag=f"d{pid}", name=f"d{pid}_{i}")
                    nc.gpsimd.tensor_tensor(out=d_[:], in0=hp_full, in1=nt_[:],
                                            op=AO.subtract)
                    e_ = gwp.tile([64, 512], bft, tag=f"e{pid}", name=f"e{pid}_{i}")
                    nc.gpsimd.tensor_tensor(out=e_[:], in0=rzt[0:64, :], in1=d_[:],
                                            op=AO.mult)
                    nc.vector.tensor_tensor(out=hsX[:, i * 512:(i + 1) * 512],
                                            in0=nt_[:], in1=e_[:], op=AO.add)

                for i in range(NS):
                    emit(0, i, hsA, 8 + i, 72 + i)
                    emit(1, i, hsB, max(i - 24, 0), 40 + i)
                    if i == 23:
                        # c0 starts for real at i=24 reading zeros from slot 23 half A
                        nc.vector.memset(hsB[:, 23 * 512:23 * 512 + 256], 0.0)

            # ---- out projection (batched into one staging buffer + one DMA) ----
            with tc.tile_pool(name="ys", bufs=1) as ysp:
                ysb = ysp.tile([128, 16384], fp32)
                for tp in range(16):
                    for ch in range(2):
                        psy = ps.tile([128, 512], fp32, tag="ps", name=f"psy{tp}_{ch}")
                        for t8 in range(8):
                            t = 8 * tp + t8
                            if t < 32:
                                hsX, col = hsB, (t + 24) * 512
                            elif t < 64:
                                hsX, col = hsA, (t - 8) * 512
                            elif t < 96:
                                hsX, col = hsB, (t - 40) * 512 + 256
                            else:
                                hsX, col = hsA, (t - 72) * 512 + 256
                            nc.tensor.matmul(
                                psy[:, 64 * t8:64 * (t8 + 1)],
                                hsX[:, col + ch * 128:col + ch * 128 + 128],
                                outwt[:], start=True, stop=True)
                        nc.vector.scalar_tensor_tensor(
                            ysb[:, (2 * tp + ch) * 512:(2 * tp + ch + 1) * 512],
                            psy[:], 1.0, outbrep[:], AO.mult, AO.add)
                # ysb layout: [(seq_chunk=128), (tp, ch, t8, w)] -> yout [256, 128, 64]
                nc.sync.dma_start(
                    out=yout_d[:].rearrange("(ch s) (tp t8) w -> s tp ch t8 w", ch=2, t8=8),
                    in_=ysb[:].rearrange("s (tp ch t8 w) -> s tp ch t8 w", tp=16, ch=2, w=64))

    _legalize_multiwaits(nc)
    return nc


_L2_CACHE = {}


def _get_l2():
    if "nc" not in _L2_CACHE:
        _L2_CACHE["nc"] = _build_l2()
    return _L2_CACHE["nc"]


def _run_l2(houts, gru_Wx, gru_Wh, gru_bx, gru_bh, out_W, out_b):
    import os
    from concourse.bass_utils import run_bass_kernel_spmd
    nc = _get_l2()
    H = 64
    zr = np.concatenate([np.arange(H, 2 * H), np.arange(0, H)])  # [z, r] order
    wxrz = np.ascontiguousarray(gru_Wx[zr, :].T).astype(bf16)
    wxn = np.ascontiguousarray(gru_Wx[2 * H:, :].T).astype(bf16)
    whrz = np.ascontiguousarray(gru_Wh[zr, :].T).astype(bf16)
    whn = np.ascontiguousarray(gru_Wh[2 * H:, :].T).astype(bf16)
    bhn = gru_bh[2 * H:].reshape(1, 64).astype(bf16)
    ones512 = np.ones((1, 512), bf16)
    brz = (gru_bx[zr] + gru_bh[zr]).reshape(128, 1).astype(np.float32)
    bxncol = gru_bx[2 * H:].reshape(64, 1).astype(np.float32)
    outwt = np.ascontiguousarray(out_W.T).astype(bf16)
    outbrep = np.tile(out_b, 8)[None, :].repeat(128, 0).astype(np.float32)

    in_maps = []
    for c in range(N_CORES):
        b, nh = c // 2, c % 2
        A = houts[2 * b][:, :, 256 * nh:256 * (nh + 1)]
        Bc = houts[2 * b + 1][:, :, 256 * nh:256 * (nh + 1)]
        hin = np.ascontiguousarray(np.concatenate([A, Bc], axis=0))     # [128,64,256]
        in_maps.append({
            "hin": hin, "wxrz": wxrz, "wxn": wxn, "whrz": whrz, "whn": whn,
            "bhn": bhn, "ones512": ones512, "brz": brz, "bxncol": bxncol,
            "outwt": outwt, "outbrep": outbrep,
        })
    trace = bool(os.environ.get("TRNK_TRACE"))
    res = run_bass_kernel_spmd(nc, in_maps, list(range(N_CORES)), trace=trace)
    _LAST_PERF["l2_ns"] = res.exec_time_ns
    _LAST_PERF["l2_trace"] = res.instructions_and_trace[1] if res.instructions_and_trace else None
    y = np.empty((B, NX, NT, W), np.float32)
    for c in range(N_CORES):
        b, nh = c // 2, c % 2
        y[b, 256 * nh:256 * (nh + 1)] = res.results[c]["yout"]
    return y


# revision 20
# speedup vs baseline: 1.0371x; 1.0371x over previous
import numpy as np
import ml_dtypes

bf16 = ml_dtypes.bfloat16

B, NX, NT, W, M = 4, 512, 128, 64, 16
N = NX
EPS = 1e-5
TWO_PI = float(2.0 * np.pi)
INV_2PI = float(1.0 / (2.0 * np.pi))
MAGIC = float(1.5 * 2 ** 23)
N_CORES = 8
WWARM = 24
T1 = (NT + WWARM) // 2  # 76; chain A: t 0..T1-1, chain B: warm T1-WWARM..T1-1, real T1..NT-1
NSEQ = 256              # sequences per core in launch 2


# ----------------------------------------------------------------------------
# toolchain workaround: this walrus build rejects instructions carrying more
# sync waits than the ISA struct holds (the Tile tail-drain gets one wait per
# engine/DMA queue). Hoist excess waits into single-wait NoOps just before.
# ----------------------------------------------------------------------------
def _legalize_multiwaits(nc):
    import concourse.mybir as mybir
    for f in nc.m.functions:
        for blk in f.blocks:
            insts = blk.instructions
            out = []
            changed = False
            for inst in insts:
                si = inst.sync_info
                limit = 2 if isinstance(inst, mybir.InstEventSemaphore) else 1
                if si is not None and len(si.on_wait) > limit:
                    waits = list(si.on_wait)
                    head, tail = waits[:-limit], waits[-limit:]
                    for k, w in enumerate(head):
                        out.append(mybir.InstNoOp(
                            name=f"{inst.name}-wsplit{k}", ins=[], outs=[],
                            engine=inst.engine,
                            sync_info=mybir.SyncInfo(on_wait=[w], on_update=[])))
                    inst.sync_info = mybir.SyncInfo(
                        on_wait=tail, on_update=list(si.on_update))
                    changed = True
                out.append(inst)
            if changed:
                blk.instructions = out


# ----------------------------------------------------------------------------
# host parameter precompute (pure numpy, cheap)
# ----------------------------------------------------------------------------
def _host_params(grid, pole_re, pole_im, res_re, res_im, conv_w, conv_b):
    g64 = grid.astype(np.float64)
    dt = g64[1] - g64[0]
    f = np.fft.fftfreq(N, dt)
    lam = 1j * 2 * np.pi * f
    w_vec = np.zeros(N, complex)
    w_vec[1:] = 1.0 / lam[1:]
    Kc = np.fft.ifft(w_vec)
    K_re = Kc.real
    altn = np.where(np.arange(N) % 2 == 0, 1.0, -1.0)
    kappa = float((Kc.imag * altn).mean())
    idx = (np.arange(N)[None, :] - np.arange(N)[:, None]) % N
    C = K_re[idx]                              # [n', n]
    s_re = N * K_re[(-np.arange(N)) % N]
    Ssum = float(s_re.sum())

    p = (pole_re + 1j * pole_im).astype(np.complex128)
    r = (res_re + 1j * res_im).astype(np.complex128)
    R1 = r.sum(-1)

    Eall = np.zeros((128, 256))
    for k in range(3):
        pk = p ** k / (1.0 if k < 2 else 2.0) if k else np.ones_like(p)
        Ak, Bk = pk.real, pk.imag
        Mre = np.einsum('jcm,com->jo', r.real, Ak) - np.einsum('jcm,com->jo', r.imag, Bk)
        Mim = -np.einsum('jcm,com->jo', r.imag, Ak) - np.einsum('jcm,com->jo', r.real, Bk)
        Eall[0:64, 64 * k:64 * k + 64] = -Mre / N
        Eall[64:128, 64 * k:64 * k + 64] = -kappa * Mim
    Eall[64:128, 192:256] = -kappa * R1.imag

    G4 = np.stack([np.ones(N), g64, g64 ** 2, altn]).astype(bf16)   # [4, N]

    blkdiag = lambda A: np.block([[A, np.zeros_like(A)], [np.zeros_like(A), A]])
    r1blk = blkdiag(R1.real).astype(bf16)                 # [(t,i), (t,o)]
    convblk = blkdiag(conv_w.T / TWO_PI).astype(bf16)     # lhsT: [i, o] blocks
    sa4 = np.zeros((N, 4), np.float32)
    sa4[:, 0] = s_re
    sa4[:, 1] = altn
    cb2pi = np.tile(conv_b.astype(np.float64) / TWO_PI, 2).reshape(128, 1)

    return dict(
        cmat=C.astype(bf16), r1blk=r1blk, convblk=convblk,
        sa4=sa4.astype(bf16), eall=Eall.astype(np.float32),
        g4=np.ascontiguousarray(G4), Ssum=Ssum,
        cb2pi=cb2pi.astype(np.float32),
        identb=np.eye(128, dtype=bf16), identf=np.eye(128, dtype=np.float32),
        epsv=np.full((128, 1), EPS, np.float32),
    )


# ----------------------------------------------------------------------------
# launch 1: spectral + conv + sin   (per core: one (b, t-half) slab)
# ----------------------------------------------------------------------------
def _build_l1(Ssum):
    import concourse.bass as bass
    import concourse.mybir as mybir
    from concourse.tile import TileContext

    fp32 = mybir.dt.float32
    bft = mybir.dt.bfloat16
    AO = mybir.AluOpType
    AF = mybir.ActivationFunctionType

    nc = bass.Bass()
    xin_d = nc.declare_dram_parameter("xin", [512, 4096], bft, isOutput=False)
    cmat_d = nc.declare_dram_parameter("cmat", [512, 512], bft, isOutput=False)
    r1blk_d = nc.declare_dram_parameter("r1blk", [128, 128], bft, isOutput=False)
    convblk_d = nc.declare_dram_parameter("convblk", [128, 128], bft, isOutput=False)
    sa4_d = nc.declare_dram_parameter("sa4", [512, 4], bft, isOutput=False)
    eall_d = nc.declare_dram_parameter("eall", [128, 256], fp32, isOutput=False)
    g4_d = nc.declare_dram_parameter("g4", [4, 512], bft, isOutput=False)
    identb_d = nc.declare_dram_parameter("identb", [128, 128], bft, isOutput=False)
    identf_d = nc.declare_dram_parameter("identf", [128, 128], fp32, isOutput=False)
    cb2pi_d = nc.declare_dram_parameter("cb2pi", [128, 1], fp32, isOutput=False)
    epsv_d = nc.declare_dram_parameter("epsv", [128, 1], fp32, isOutput=False)
    hout_d = nc.declare_dram_parameter("hout", [64, 64, 512], bft, isOutput=True)

    with TileContext(nc) as tc:
        with tc.tile_pool(name="const", bufs=1) as cp, \
             tc.tile_pool(name="xa", bufs=1) as xap, \
             tc.tile_pool(name="big", bufs=1) as bigp, \
             tc.tile_pool(name="st", bufs=1) as stp, \
             tc.tile_pool(name="wk", bufs=3) as wkp, \
             tc.tile_pool(name="scr", bufs=2) as scrp, \
             tc.tile_pool(name="ps", bufs=8, space="PSUM") as ps:

            # ---- constants in ----
            cm = [cp.tile([128, 512], bft, tag=f"cm{c}", name=f"cm{c}") for c in range(4)]
            for c in range(4):
                nc.sync.dma_start(out=cm[c][:], in_=cmat_d[128 * c:128 * (c + 1), :])
            r1blk = cp.tile([128, 128], bft)
            nc.sync.dma_start(out=r1blk[:], in_=r1blk_d[:])
            convblk = cp.tile([128, 128], bft)
            nc.sync.dma_start(out=convblk[:], in_=convblk_d[:])
            sa4 = [cp.tile([128, 4], bft, tag=f"sa{c}", name=f"sa{c}") for c in range(4)]
            for c in range(4):
                nc.sync.dma_start(out=sa4[c][:], in_=sa4_d[128 * c:128 * (c + 1), :])
            eall = cp.tile([128, 256], fp32)
            nc.sync.dma_start(out=eall[:], in_=eall_d[:])
            g4 = cp.tile([4, 512], bft)
            nc.sync.dma_start(out=g4[:], in_=g4_d[:])
            identb = cp.tile([128, 128], bft)
            nc.sync.dma_start(out=identb[:], in_=identb_d[:])
            identf = cp.tile([128, 128], fp32)
            nc.sync.dma_start(out=identf[:], in_=identf_d[:])
            cb2pi = cp.tile([128, 1], fp32)
            nc.sync.dma_start(out=cb2pi[:], in_=cb2pi_d[:])
            epsv = cp.tile([128, 1], fp32)
            nc.sync.dma_start(out=epsv[:], in_=epsv_d[:])

            # ---- x in (bf16, layout A: [nx, (t,w)]) ----
            xa = [xap.tile([128, 4096], bft, tag=f"xa{c}", name=f"xa{c}") for c in range(4)]
            for c in range(4):
                nc.sync.dma_start(out=xa[c][:], in_=xin_d[128 * c:128 * (c + 1), :])

            # ---- S/alt matvec: out rows [Sx, Xalt, -, -] ----
            sxrow = stp.tile([4, 4096], fp32)
            for ch in range(8):
                pssx = ps.tile([4, 512], fp32, tag="ps", name="pssx")
                for c in range(4):
                    nc.tensor.matmul(pssx[:], sa4[c][:],
                                     xa[c][:, 512 * ch:512 * (ch + 1)],
                                     start=(c == 0), stop=(c == 3))
                nc.vector.tensor_copy(sxrow[:, 512 * ch:512 * (ch + 1)], pssx[:])

            # ---- transpose to layout B + norm1 stats + drains (groups of 8) ----
            ub = [bigp.tile([128, 512], bft, tag=f"ub{j}", name=f"ub{j}") for j in range(32)]
            xr = [bigp.tile([128, 512], bft, tag=f"xr{j}", name=f"xr{j}") for j in range(32)]
            sum1 = stp.tile([128, 32], fp32)
            ssq1 = stp.tile([128, 32], fp32)
            statpack = stp.tile([128, 64], fp32)   # cols 0:32 rs1, 32:64 negms1
            rs1 = statpack[:, 0:32]
            negms1 = statpack[:, 32:64]
            mean1 = stp.tile([128, 32], fp32)
            tmpa = stp.tile([128, 32], fp32)

            for g in range(4):
                js = range(8 * g, 8 * g + 8)
                ptw = {}
                for j in js:
                    ptw[j] = ps.tile([128, 512], bft, tag="ps", name=f"ptw{j}")
                    for c in range(4):
                        nc.tensor.transpose(ptw[j][:, 128 * c:128 * (c + 1)],
                                            xa[c][:, 128 * j:128 * (j + 1)],
                                            identb[:])
                for j in js:
                    nc.vector.tensor_scalar(
                        xr[j][:], ptw[j][:], 1.0, 0.0, AO.mult, AO.add,
                        accum_out=sum1[:, j:j + 1])
                    sq = scrp.tile([128, 512], bft, tag="sq", name="sq")
                    nc.scalar.activation(sq[:], ptw[j][:], AF.Square,
                                         accum_out=ssq1[:, j:j + 1])
                gs = slice(8 * g, 8 * g + 8)
                nc.vector.tensor_scalar(mean1[:, gs], sum1[:, gs],
                                        1.0 / 512.0, None, AO.mult)
                nc.vector.scalar_tensor_tensor(tmpa[:, gs], mean1[:, gs], 1.0,
                                               mean1[:, gs], AO.mult, AO.mult)
                nc.vector.scalar_tensor_tensor(tmpa[:, gs], ssq1[:, gs],
                                               1.0 / 512.0, tmpa[:, gs],
                                               AO.mult, AO.subtract)
                nc.scalar.activation(tmpa[:, gs], tmpa[:, gs], AF.Sqrt,
                                     bias=epsv[:, 0:1])
                nc.vector.reciprocal(rs1[:, gs], tmpa[:, gs])
                nc.vector.scalar_tensor_tensor(negms1[:, gs], mean1[:, gs], -1.0,
                                               rs1[:, gs], AO.mult, AO.mult)
                for j in js:
                    nc.scalar.activation(ub[j][:], ptw[j][:], AF.Identity,
                                         bias=negms1[:, j:j + 1],
                                         scale=rs1[:, j:j + 1])

            # ---- S-fix and D coefficients ----
            pst = ps.tile([64, 128], fp32, tag="ps", name="pst")
            nc.tensor.transpose(pst[:], statpack[:], identf[:])
            statT = stp.tile([64, 128], fp32)
            nc.vector.tensor_copy(statT[:], pst[:])

            rs1m2 = stp.tile([64, 128], fp32)
            negms2m = stp.tile([64, 128], fp32)
            # statT[q, p] = statpack[p, q]: q<32 -> rs1 tile q; q>=32 -> negms tile q-32
            # rs1m[t=2q+tt, w] = statT[q, 64*tt + w]
            for half in range(2):
                nc.sync.dma_start(
                    out=rs1m2[:, 64 * half:64 * half + 64],
                    in_=statT[0:32, :].rearrange("q (tt w) -> q tt w", tt=2))
            nc.sync.dma_start(
                out=negms2m[:, 0:64],
                in_=statT[32:64, :].rearrange("q (tt w) -> q tt w", tt=2))
            nc.vector.memset(negms2m[:, 64:128], 0.0)

            Sm = stp.tile([64, 128], fp32)
            nc.sync.dma_start(out=Sm[:, 0:64],
                              in_=sxrow[0:1, :].rearrange("o (t j) -> o t j", t=64))
            nc.sync.dma_start(out=Sm[:, 64:128],
                              in_=sxrow[1:2, :].rearrange("o (t j) -> o t j", t=64))
            nc.vector.tensor_tensor(out=Sm[:], in0=Sm[:], in1=rs1m2[:], op=AO.mult)
            nc.vector.scalar_tensor_tensor(Sm[:], negms2m[:], Ssum, Sm[:],
                                           AO.mult, AO.add)
            psu = ps.tile([128, 64], fp32, tag="ps", name="psu")
            nc.tensor.transpose(psu[:], Sm[:], identf[0:64, 0:64])
            suT = stp.tile([128, 64], fp32)
            nc.vector.tensor_copy(suT[:], psu[:])
            psd = ps.tile([64, 256], fp32, tag="ps", name="psd")
            nc.tensor.matmul(psd[:], suT[:], eall[:], start=True, stop=True)
            dallf = stp.tile([64, 256], fp32)
            nc.vector.tensor_copy(dallf[:], psd[:])
            dallb = stp.tile([64, 256], bft)
            nc.vector.tensor_copy(dallb[:], dallf[:])
            dco = stp.tile([4, 4096], bft)
            for k in range(4):
                nc.sync.dma_start(
                    out=dco[k:k + 1, :],
                    in_=dallb[:, 64 * k:64 * k + 64])

            # ---- channel mix (P^T direct) ----
            pt = [bigp.tile([128, 4096], bft, tag=f"pt{c}", name=f"pt{c}") for c in range(4)]
            for c in range(4):
                for jq in range(8):
                    psc = ps.tile([128, 512], fp32, tag="ps", name="psc")
                    for jj in range(4):
                        j = 4 * jq + jj
                        nc.tensor.matmul(psc[:, 128 * jj:128 * (jj + 1)],
                                         ub[j][:, 128 * c:128 * (c + 1)],
                                         r1blk[:], start=True, stop=True)
                    nc.vector.tensor_copy(pt[c][:, 512 * jq:512 * (jq + 1)], psc[:])

            # ---- time mix + rank + conv + norm2 + sin (groups of 6) ----
            mv2 = stp.tile([128, 64], fp32)      # (mean, var) pairs per tile
            rs2p = stp.tile([128, 32], fp32)
            cc2 = stp.tile([128, 32], fp32)
            tmpb = stp.tile([128, 32], fp32)
            h2ref = [None]
            groups = [list(range(6 * g, min(6 * g + 6, 32))) for g in range(6)]
            for js in groups:
                px1 = {}
                for j in js:
                    px1[j] = ps.tile([128, 512], fp32, tag="ps", name=f"px1_{j}")
                    for c in range(4):
                        nc.tensor.matmul(px1[j][:], pt[c][:, 128 * j:128 * (j + 1)],
                                         cm[c][:], start=(c == 0), stop=False)
                    nc.tensor.matmul(px1[j][:], dco[:, 128 * j:128 * (j + 1)],
                                     g4[:], start=False, stop=True)
                    bns = scrp.tile([128, 6], fp32, tag="bns", name="bns")
                    nc.vector.bn_stats(bns[:], px1[j][:])
                    nc.vector.bn_aggr(mv2[:, 2 * j:2 * j + 2], bns[:])
                g0, g1 = js[0], js[-1] + 1
                gs = slice(g0, g1)
                nc.scalar.activation(tmpb[:, gs], mv2[:, 2 * g0 + 1:2 * g1:2],
                                     AF.Sqrt, bias=epsv[:, 0:1])
                nc.vector.reciprocal(tmpb[:, gs], tmpb[:, gs])
                nc.vector.tensor_scalar(rs2p[:, gs], tmpb[:, gs], INV_2PI, None,
                                        AO.mult)
                nc.vector.scalar_tensor_tensor(tmpb[:, gs], mv2[:, 2 * g0:2 * g1:2],
                                               -1.0, rs2p[:, gs], AO.mult, AO.mult)
                nc.vector.tensor_scalar(cc2[:, gs], tmpb[:, gs], cb2pi[:, 0:1],
                                        None, AO.add)
                for j in js:
                    pcv = ps.tile([128, 512], fp32, tag="ps", name="pcv")
                    nc.tensor.matmul(pcv[:], convblk[:], xr[j][:],
                                     start=True, stop=True)
                    qa = wkp.tile([128, 512], fp32, tag="qa", name="qa")
                    nc.scalar.activation(qa[:], px1[j][:], AF.Identity,
                                         bias=cc2[:, j:j + 1],
                                         scale=rs2p[:, j:j + 1])
                    qq = wkp.tile([128, 512], fp32, tag="qq", name="qq")
                    nc.vector.scalar_tensor_tensor(qq[:], pcv[:], 1.0, qa[:],
                                                   AO.mult, AO.add)
                    kk = wkp.tile([128, 512], fp32, tag="kk", name="kk")
                    nc.gpsimd.tensor_scalar(kk[:], qq[:], MAGIC, MAGIC,
                                            AO.add, AO.subtract)
                    rr = wkp.tile([128, 512], fp32, tag="rr", name="rr")
                    nc.gpsimd.tensor_tensor(out=rr[:], in0=qq[:], in1=kk[:],
                                            op=AO.subtract)
                    if j % 4 == 0:
                        h2ref[0] = wkp.tile([128, 2048], bft, tag="h2",
                                            name=f"h2_{j}", bufs=2)
                    h2 = h2ref[0]
                    nc.scalar.activation(h2[:, (j % 4) * 512:(j % 4) * 512 + 512],
                                         rr[:], AF.Sin, scale=TWO_PI)
                    if j % 4 == 3:
                        j0 = j - 3
                        nc.sync.dma_start(
                            out=hout_d[2 * j0:2 * j0 + 8, :, :].rearrange(
                                "(jj tt) w n -> tt w jj n", jj=4),
                            in_=h2[:].rearrange("p (jj n) -> p jj n", jj=4))

    _legalize_multiwaits(nc)
    return nc


_L1_CACHE = {}


def _get_l1(Ssum):
    key = round(Ssum, 9)
    if key not in _L1_CACHE:
        _L1_CACHE[key] = _build_l1(Ssum)
    return _L1_CACHE[key]


_LAST_PERF = {}


def _run_l1(x, prm):
    import os
    from concourse.bass_utils import run_bass_kernel_spmd
    nc = _get_l1(prm['Ssum'])
    in_maps = []
    xb = x.astype(bf16)
    for c in range(N_CORES):
        b, th = c // 2, c % 2
        slab = np.ascontiguousarray(xb[b, :, 64 * th:64 * (th + 1), :]).reshape(512, 4096)
        m = {"xin": slab}
        for k in ("cmat", "r1blk", "convblk", "sa4", "eall", "g4", "identb",
                  "identf", "cb2pi", "epsv"):
            m[k] = prm[k]
        in_maps.append(m)
    trace = bool(os.environ.get("TRNK_TRACE"))
    res = run_bass_kernel_spmd(nc, in_maps, list(range(N_CORES)), trace=trace)
    _LAST_PERF["l1_ns"] = res.exec_time_ns
    _LAST_PERF["l1_trace"] = res.instructions_and_trace[1] if res.instructions_and_trace else None
    return [r["hout"] for r in res.results]


# ----------------------------------------------------------------------------
# reference-path helpers for the final host fallback (and debugging)
# ----------------------------------------------------------------------------
def _instance_norm(v):
    mu = v.mean(-1, keepdims=True)
    var = ((v - mu) ** 2).mean(-1, keepdims=True)
    return (v - mu) / np.sqrt(var + EPS)


def _host_launch1(x, prm, conv_w, conv_b):
    xs = np.ascontiguousarray(x.transpose(0, 2, 3, 1)).reshape(B * NT, W, NX)
    xb = xs.astype(bf16).astype(np.float32)
    mu = xb.mean(-1, keepdims=True)
    var = (xb ** 2).mean(-1, keepdims=True) - mu ** 2
    rs = 1.0 / np.sqrt(var + EPS)
    u = ((xb - mu) * rs).astype(bf16).astype(np.float32)
    R1b = prm['r1blk'][:64, :64].astype(np.float32)
    PT = np.einsum('bin,io->bon', u, R1b).astype(bf16).astype(np.float32)
    x1a = np.einsum('bon,nz->boz', PT, prm['cmat'].astype(np.float32))
    sre = prm['sa4'][:, 0].astype(np.float32)
    alt = prm['sa4'][:, 1].astype(np.float32)
    Sx = np.einsum('bjn,n->bj', xb, sre)
    Xa = np.einsum('bjn,n->bj', xb, alt)
    S = rs[:, :, 0] * Sx + prm['Ssum'] * (-mu[:, :, 0] * rs[:, :, 0])
    ua = rs[:, :, 0] * Xa
    SU = np.concatenate([S, ua], 1)
    Dall = (SU @ prm['eall']).astype(bf16).astype(np.float32)
    G4 = prm['g4'].astype(np.float32)
    x2 = np.einsum('btk,kn->btn',
                   Dall.reshape(-1, 4, 64).transpose(0, 2, 1), G4)
    x1f = x1a + x2
    mu2 = x1f.mean(-1, keepdims=True)
    var2 = (x1f ** 2).mean(-1, keepdims=True) - mu2 ** 2
    rs2 = 1.0 / np.sqrt(var2 + EPS)
    convb = np.einsum('oi,bin->bon',
                      prm['convblk'][:64, :64].T.astype(np.float32), xb)
    qa = x1f * (rs2 * INV_2PI) + (-mu2 * rs2 + conv_b[None, :, None]) * INV_2PI
    q = (qa + convb).astype(np.float32)
    kq = ((q + np.float32(MAGIC)) - np.float32(MAGIC)).astype(np.float32)
    h = np.sin(TWO_PI * (q - kq)).astype(bf16).astype(np.float32)
    return h  # [BT, W, NX]


def kernel(x, grid, pole_re, pole_im, res_re, res_im, conv_w, conv_b,
           gru_Wx, gru_Wh, gru_bx, gru_bh, out_W, out_b):
    x = np.asarray(x, np.float32)
    grid = np.asarray(grid, np.float32)
    prm = _host_params(grid, np.asarray(pole_re), np.asarray(pole_im),
                       np.asarray(res_re), np.asarray(res_im),
                       np.asarray(conv_w, np.float32), np.asarray(conv_b, np.float32))
    houts = _run_l1(x, prm)   # 8 x [64, 64, 512] bf16
    y = _run_l2(houts, np.asarray(gru_Wx, np.float32), np.asarray(gru_Wh, np.float32),
                np.asarray(gru_bx, np.float32), np.asarray(gru_bh, np.float32),
                np.asarray(out_W, np.float32), np.asarray(out_b, np.float32))
    return y


# ----------------------------------------------------------------------------
# launch 2: GRU (2 time-chunks, alternating) + output projection
# ----------------------------------------------------------------------------
def _build_l2():
    import concourse.bass as bass
    import concourse.mybir as mybir
    from concourse.tile import TileContext

    fp32 = mybir.dt.float32
    bft = mybir.dt.bfloat16
    AO = mybir.AluOpType
    AF = mybir.ActivationFunctionType
    T = NT
    TOFF = T - T1  # 52: chain-B time offset

    nc = bass.Bass()
    hin_d = nc.declare_dram_parameter("hin", [128, 64, 256], bft, isOutput=False)
    wxrz_d = nc.declare_dram_parameter("wxrz", [64, 128], bft, isOutput=False)
    wxn_d = nc.declare_dram_parameter("wxn", [64, 64], bft, isOutput=False)
    whrz_d = nc.declare_dram_parameter("whrz", [64, 128], bft, isOutput=False)
    whn_d = nc.declare_dram_parameter("whn", [64, 64], bft, isOutput=False)
    bhn_d = nc.declare_dram_parameter("bhn", [1, 64], bft, isOutput=False)
    ones_d = nc.declare_dram_parameter("ones512", [1, 512], bft, isOutput=False)
    brz_d = nc.declare_dram_parameter("brz", [128, 1], fp32, isOutput=False)
    bxn_d = nc.declare_dram_parameter("bxncol", [64, 1], fp32, isOutput=False)
    outwt_d = nc.declare_dram_parameter("outwt", [64, 64], bft, isOutput=False)
    outbrep_d = nc.declare_dram_parameter("outbrep", [128, 512], fp32, isOutput=False)
    yout_d = nc.declare_dram_parameter("yout", [256, 128, 64], fp32, isOutput=True)

    with TileContext(nc) as tc:
        with tc.tile_pool(name="const", bufs=1) as cp, \
             tc.tile_pool(name="hs", bufs=1) as hsp, \
             tc.tile_pool(name="gw", bufs=2) as gwp, \
             tc.tile_pool(name="ps", bufs=8, space="PSUM") as ps:

            wxrz = cp.tile([64, 128], bft)
            nc.sync.dma_start(out=wxrz[:], in_=wxrz_d[:])
            wxn = cp.tile([64, 64], bft)
            nc.sync.dma_start(out=wxn[:], in_=wxn_d[:])
            whrz = cp.tile([64, 128], bft)
            nc.sync.dma_start(out=whrz[:], in_=whrz_d[:])
            whn = cp.tile([64, 64], bft)
            nc.sync.dma_start(out=whn[:], in_=whn_d[:])
            bhn = cp.tile([1, 64], bft)
            nc.sync.dma_start(out=bhn[:], in_=bhn_d[:])
            ones512 = cp.tile([1, 512], bft)
            nc.sync.dma_start(out=ones512[:], in_=ones_d[:])
            brz = cp.tile([128, 1], fp32)
            nc.sync.dma_start(out=brz[:], in_=brz_d[:])
            bxnc = cp.tile([64, 1], fp32)
            nc.sync.dma_start(out=bxnc[:], in_=bxn_d[:])
            outwt = cp.tile([64, 64], bft)
            nc.sync.dma_start(out=outwt[:], in_=outwt_d[:])
            outbrep = cp.tile([128, 512], fp32)
            nc.sync.dma_start(out=outbrep[:], in_=outbrep_d[:])

            NS = 56  # steps per pair
            hsA = hsp.tile([64, 512 * NS], bft)   # pair A = (c1: t=8+i, c3: t=72+i)
            hsB = hsp.tile([64, 512 * NS], bft)   # pair B = (c0: t=i-24, c2: t=40+i)
            hz = cp.tile([64, 512], bft)
            nc.vector.memset(hz[:], 0.0)

            with tc.tile_pool(name="hin", bufs=1) as hinp:
                hin_all = hinp.tile([64, 32768], bft)
                nc.sync.dma_start(
                    out=hin_all[:].rearrange("w (t n) -> w t n", t=128),
                    in_=hin_d[:].rearrange("t w n -> w t n"))

                def emit(pid, i, hsX, tA, tB):
                    if i == 0:
                        hpA, hpB = hz[:, 0:256], hz[:, 256:512]
                        hp_full = hz[:]
                    else:
                        base = (i - 1) * 512
                        hpA = hsX[:, base:base + 256]
                        hpB = hsX[:, base + 256:base + 512]
                        hp_full = hsX[:, base:base + 512]
                    hA = hin_all[:, tA * 256:(tA + 1) * 256]
                    hB = hin_all[:, tB * 256:(tB + 1) * 256]
                    psA = ps.tile([128, 512], fp32, tag="ps", name=f"psA{pid}_{i}")
                    # group-by-group: a start=True clears the whole bank's
                    # has_written bits, so finish half A before starting half B
                    nc.tensor.matmul(psA[:, 0:256], whrz[:], hpA, start=True, stop=False)
                    nc.tensor.matmul(psA[:, 0:256], wxrz[:], hA, start=False, stop=True)
                    nc.tensor.matmul(psA[:, 256:512], whrz[:], hpB, start=True, stop=False)
                    nc.tensor.matmul(psA[:, 256:512], wxrz[:], hB, start=False, stop=True)
                    rzt = gwp.tile([128, 512], bft, tag=f"rzt{pid}", name=f"rzt{pid}_{i}")
                    nc.scalar.activation(rzt[:], psA[:], AF.Sigmoid, bias=brz[:, 0:1])
                    psB = ps.tile([64, 512], fp32, tag="ps", name=f"psB{pid}_{i}")
                    nc.tensor.matmul(psB[:], bhn[:], ones512[:], start=True, stop=False)
                    nc.tensor.matmul(psB[:, 0:256], whn[:], hpA, start=False, stop=True)
                    nc.tensor.matmul(psB[:, 256:512], whn[:], hpB, start=False, stop=True)
                    psC = ps.tile([64, 512], fp32, tag="ps", name=f"psC{pid}_{i}")
                    nc.tensor.matmul(psC[:, 0:256], wxn[:], hA, start=True, stop=True)
                    nc.tensor.matmul(psC[:, 256:512], wxn[:], hB, start=True, stop=True)
                    t1 = gwp.tile([64, 512], bft, tag=f"t1{pid}", name=f"t1_{pid}_{i}")
                    nc.vector.tensor_tensor(out=t1[:], in0=rzt[64:128, :], in1=psB[:],
                                            op=AO.mult)
                    npre = gwp.tile([64, 512], bft, tag=f"np{pid}", name=f"np{pid}_{i}")
                    nc.vector.tensor_tensor(out=npre[:], in0=t1[:], in1=psC[:],
                                            op=AO.add)
                    nt_ = gwp.tile([64, 512], bft, tag=f"nt{pid}", name=f"nt{pid}_{i}")
                    nc.scalar.activation(nt_[:], npre[:], AF.Tanh, bias=bxnc[:, 0:1])
                    d_ = gwp.tile([64, 512], bft, tag=f"d{pid}", name=f"d{pid}_{i}")
                    nc.gpsimd.tensor_tensor(out=d_[:], in0=hp_full, in1=nt_[:],
                                            op=AO.subtract)
                    e_ = gwp.tile([64, 512], bft, tag=f"e{pid}", name=f"e{pid}_{i}")
                    nc.gpsimd.tensor_tensor(out=e_[:], in0=rzt[0:64, :], in1=d_[:],
                                            op=AO.mult)
                    nc.vector.tensor_tensor(out=hsX[:, i * 512:(i + 1) * 512],
                                            in0=nt_[:], in1=e_[:], op=AO.add)

                for i in range(NS):
                    emit(0, i, hsA, 8 + i, 72 + i)
                    emit(1, i, hsB, max(i - 24, 0), 40 + i)
                    if i == 23:
                        # c0 starts for real at i=24 reading zeros from slot 23 half A
                        nc.vector.memset(hsB[:, 23 * 512:23 * 512 + 256], 0.0)

            # ---- out projection (batched into one staging buffer + one DMA) ----
            with tc.tile_pool(name="ys", bufs=1) as ysp:
                ysb = ysp.tile([128, 16384], fp32)
                for tp in range(16):
                    for ch in range(2):
                        psy = ps.tile([128, 512], fp32, tag="ps", name=f"psy{tp}_{ch}")
                        for t8 in range(8):
                            t = 8 * tp + t8
                            if t < 32:
                                hsX, col = hsB, (t + 24) * 512
                            elif t < 64:
                                hsX, col = hsA, (t - 8) * 512
                            elif t < 96:
                                hsX, col = hsB, (t - 40) * 512 + 256
                            else:
                                hsX, col = hsA, (t - 72) * 512 + 256
                            nc.tensor.matmul(
                                psy[:, 64 * t8:64 * (t8 + 1)],
                                hsX[:, col + ch * 128:col + ch * 128 + 128],
                                outwt[:], start=True, stop=True)
                        nc.vector.scalar_tensor_tensor(
                            ysb[:, (2 * tp + ch) * 512:(2 * tp + ch + 1) * 512],
                            psy[:], 1.0, outbrep[:], AO.mult, AO.add)
                # ysb layout: [(seq_chunk=128), (tp, ch, t8, w)] -> yout [256, 128, 64]
                nc.sync.dma_start(
                    out=yout_d[:].rearrange("(ch s) (tp t8) w -> s tp ch t8 w", ch=2, t8=8),
                    in_=ysb[:].rearrange("s (tp ch t8 w) -> s tp ch t8 w", tp=16, ch=2, w=64))

    _legalize_multiwaits(nc)
    return nc


_L2_CACHE = {}


def _get_l2():
    if "nc" not in _L2_CACHE:
        _L2_CACHE["nc"] = _build_l2()
    return _L2_CACHE["nc"]


def _run_l2(houts, gru_Wx, gru_Wh, gru_bx, gru_bh, out_W, out_b):
    import os
    from concourse.bass_utils import run_bass_kernel_spmd
    nc = _get_l2()
    H = 64
    zr = np.concatenate([np.arange(H, 2 * H), np.arange(0, H)])  # [z, r] order
    wxrz = np.ascontiguousarray(gru_Wx[zr, :].T).astype(bf16)
    wxn = np.ascontiguousarray(gru_Wx[2 * H:, :].T).astype(bf16)
    whrz = np.ascontiguousarray(gru_Wh[zr, :].T).astype(bf16)
    whn = np.ascontiguousarray(gru_Wh[2 * H:, :].T).astype(bf16)
    bhn = gru_bh[2 * H:].reshape(1, 64).astype(bf16)
    ones512 = np.ones((1, 512), bf16)
    brz = (gru_bx[zr] + gru_bh[zr]).reshape(128, 1).astype(np.float32)
    bxncol = gru_bx[2 * H:].reshape(64, 1).astype(np.float32)
    outwt = np.ascontiguousarray(out_W.T).astype(bf16)
    outbrep = np.tile(out_b, 8)[None, :].repeat(128, 0).astype(np.float32)

    in_maps = []
    for c in range(N_CORES):
        b, nh = c // 2, c % 2
        A = houts[2 * b][:, :, 256 * nh:256 * (nh + 1)]
        Bc = houts[2 * b + 1][:, :, 256 * nh:256 * (nh + 1)]
        hin = np.ascontiguousarray(np.concatenate([A, Bc], axis=0))     # [128,64,256]
        in_maps.append({
            "hin": hin, "wxrz": wxrz, "wxn": wxn, "whrz": whrz, "whn": whn,
            "bhn": bhn, "ones512": ones512, "brz": brz, "bxncol": bxncol,
            "outwt": outwt, "outbrep": outbrep,
        })
    trace = bool(os.environ.get("TRNK_TRACE"))
    res = run_bass_kernel_spmd(nc, in_maps, list(range(N_CORES)), trace=trace)
    _LAST_PERF["l2_ns"] = res.exec_time_ns
    _LAST_PERF["l2_trace"] = res.instructions_and_trace[1] if res.instructions_and_trace else None
    y = np.empty((B, NX, NT, W), np.float32)
    for c in range(N_CORES):
        b, nh = c // 2, c % 2
        y[b, 256 * nh:256 * (nh + 1)] = res.results[c]["yout"]
    return y
